# revision 4
# baseline (speedup 1.0000x reference)
"""Trainium2 Bass kernel for CustomMHA (B=2, N=2048, D=1024, H=16, fp32).

Sharding: 8 cores = (batch b = core//4) x (head-group g = core%4, 4 heads each).
Each core computes, for its batch and its 4 heads:
    attn_out_heads @ Wout[rows of its heads]  ->  a partial [N, D] output.
Host sums the 4 partials per batch (Megatron-style row-parallel output).

Per-core pipeline (all matmuls in fp32r = TF32-like, fp32 accumulate):
  1. PE-transpose x_q, x_kv -> xT [D, N] slabs.
  2. Projections: qT/kT in [d_head, N] (transposed) layout, v in [N, d_head]
     natural layout with per-head ones/zeros-padded columns (vpo) so the
     AV matmul also produces the softmax denominator row.
  3. Attention per head, scoresT orientation [key-part, query-free]:
     QK matmul -> PSUM, exp on ACT (scale=1/8 folded) -> probsT (f32r),
     AV matmul accumulates over key blocks.  Denominator row broadcast via
     a K=1 matmul with a ones column; normalize with one DVE multiply.
  4. Row-sharded Wout matmul -> partial output.
"""

import sys

sys.path.insert(0, "/opt/trn_rl_repo")

import numpy as np

import concourse.bass as bass
import concourse.mybir as mybir
import concourse.tile as tile
from concourse import bacc
from concourse.bass_utils import run_bass_kernel_spmd
from concourse.masks import make_identity

F32 = mybir.dt.float32
F32R = mybir.dt.float32r
EXP = mybir.ActivationFunctionType.Exp

N = 2048  # sequence length
D = 1024  # model dim
HL = 4    # heads per core
O = HL * 64  # per-core projection width (256)
P = 128
NSLAB = 256          # i/j rows per transpose+projection slab
NSLABS = N // NSLAB  # 8
IG = 1024            # attention query-column group
NJB = N // P         # 16 key blocks
DC = D // P          # 8 contraction chunks


def build():
    nc = bacc.Bacc("TRN2", debug=False, num_devices=8)
    xq = nc.dram_tensor("xq", [N, D], F32, kind="ExternalInput").ap()
    xkv = nc.dram_tensor("xkv", [N, D], F32, kind="ExternalInput").ap()
    wq = nc.dram_tensor("wq", [D, O], F32, kind="ExternalInput").ap()
    wk = nc.dram_tensor("wk", [D, O], F32, kind="ExternalInput").ap()
    wv = nc.dram_tensor("wv", [D, O], F32, kind="ExternalInput").ap()
    wout = nc.dram_tensor("wout", [O, D], F32, kind="ExternalInput").ap()
    out = nc.dram_tensor("out", [N, D], F32, kind="ExternalOutput").ap()

    with tile.TileContext(nc) as tc:
        with (
            tc.tile_pool(name="consts", bufs=1) as consts,
            tc.tile_pool(name="weights", bufs=1) as wpool,
            tc.tile_pool(name="wstage", bufs=1) as wstage,
            tc.tile_pool(name="xstage", bufs=2) as xstage,
            tc.tile_pool(name="xT", bufs=2) as xTpool,
            tc.tile_pool(name="proj", bufs=1) as projpool,
            tc.tile_pool(name="probs", bufs=2) as probspool,
            tc.tile_pool(name="bc", bufs=2) as bcpool,
            tc.tile_pool(name="ostage", bufs=2) as opool,
            tc.tile_pool(name="psum", bufs=2, space="PSUM") as psum,
        ):
            # ---- constants ----
            ident = consts.tile([P, P], F32)
            make_identity(nc, ident[:])
            onesf = consts.tile([P, P], F32)
            nc.vector.memset(onesf[:], 1.0)
            onesr = consts.tile([P, P], F32R)
            nc.vector.tensor_copy(onesr[:], onesf[:])
            # pat: [1, 0, 0, ...] column pattern for vpo padding halves
            pat = consts.tile([P, 64], F32)
            nc.vector.memset(pat[:, 0:1], 1.0)
            nc.vector.memset(pat[:, 1:64], 0.0)

            # ---- weights: DMA fp32, round to f32r ----
            with nc.named_scope("weights"):
                wr = {}
                for name, w in (("wq", wq), ("wk", wk), ("wv", wv)):
                    wt = wpool.tile([P, DC, O], F32R, tag=f"{name}r")
                    wst = wstage.tile([P, DC, O], F32, tag="wstage")
                    nc.sync.dma_start(wst[:], w.rearrange("(c p) o -> p c o", p=P))
                    nc.vector.tensor_copy(wt[:], wst[:])
                    wr[name] = wt
                woutr = wpool.tile([P, 2, D], F32R, tag="woutr")
                wst = wstage.tile([P, 2, D], F32, tag="wstage")
                nc.sync.dma_start(wst[:], wout.rearrange("(c p) o -> p c o", p=P))
                nc.vector.tensor_copy(woutr[:], wst[:])

            # ---- persistent activations ----
            qpT = projpool.tile([P, 2, N], F32R, tag="qpT")
            kpT = projpool.tile([P, 2, N], F32R, tag="kpT")
            vpo = [
                projpool.tile([P, NJB, P], F32R, tag=f"vpo{h}", name=f"vpo{h}")
                for h in range(HL)
            ]
            attT = projpool.tile([P, 2, N], F32R, tag="attT")

            # vpo padding halves: ones column + zeros
            for h in range(HL):
                pad0 = 64 if h % 2 == 0 else 0
                nc.vector.tensor_copy(
                    vpo[h][:, :, pad0 : pad0 + 64],
                    pat[:, None, :].to_broadcast([P, NJB, 64]),
                )

            # ---- transpose + projections, slab by slab ----
            with nc.named_scope("proj"):
                for s in range(NSLABS):
                    slabs = {}
                    for xin, tag in ((xq, "q"), (xkv, "kv")):
                        slab = xTpool.tile([P, DC, NSLAB], F32R, tag=f"xT{tag}")
                        for half in range(NSLAB // P):
                            ib = s * (NSLAB // P) + half
                            xt = xstage.tile([P, D], F32, tag="xstage")
                            nc.sync.dma_start(xt[:], xin[ib * P : (ib + 1) * P, :])
                            tp = psum.tile([P, DC * P], F32, tag="pA")
                            for dc in range(DC):
                                nc.tensor.transpose(
                                    tp[:, dc * P : (dc + 1) * P],
                                    xt[:, dc * P : (dc + 1) * P],
                                    ident[:],
                                )
                            # one batched PSUM->SBUF copy per i-block (rounds to f32r)
                            nc.any.tensor_copy(
                                slab[:, :, half * P : (half + 1) * P],
                                tp.rearrange("p (c i) -> p c i", c=DC),
                            )
                        slabs[tag] = slab

                    ssl = slice(s * NSLAB, (s + 1) * NSLAB)
                    for tag, wname, dstT in (("q", "wq", qpT), ("kv", "wk", kpT)):
                        for oc in range(2):
                            ps = psum.tile([P, NSLAB], F32, tag="pB")
                            for dc in range(DC):
                                nc.tensor.matmul(
                                    ps[:],
                                    wr[wname][:, dc, oc * P : (oc + 1) * P],
                                    slabs[tag][:, dc, :],
                                    start=(dc == 0),
                                    stop=(dc == DC - 1),
                                )
                            nc.any.tensor_copy(dstT[:, oc, ssl], ps[:])
                    # v projection (natural layout) + scatter into vpo
                    for half in range(NSLAB // P):
                        jb = s * (NSLAB // P) + half
                        ps = psum.tile([P, O], F32, tag="pB")
                        for dc in range(DC):
                            nc.tensor.matmul(
                                ps[:],
                                slabs["kv"][:, dc, half * P : (half + 1) * P],
                                wr["wv"][:, dc, :],
                                start=(dc == 0),
                                stop=(dc == DC - 1),
                            )
                        for h in range(HL):
                            v0 = 0 if h % 2 == 0 else 64
                            nc.any.tensor_copy(
                                vpo[h][:, jb, v0 : v0 + 64],
                                ps[:, h * 64 : (h + 1) * 64],
                            )

            # ---- attention ----
            with nc.named_scope("attention"):
                for h in range(HL):
                    oc, row0 = h // 2, (h % 2) * 64
                    vrow0 = (h % 2) * 64
                    srow = 64 - vrow0
                    for ig in range(N // IG):
                        i0 = ig * IG
                        av = psum.tile([P, IG], F32, tag="pB")
                        for jb in range(NJB):
                            qk = psum.tile([P, IG], F32, tag="pA")
                            for nb in range(IG // 512):
                                nc.tensor.matmul(
                                    qk[:, nb * 512 : (nb + 1) * 512],
                                    kpT[row0 : row0 + 64, oc, jb * P : (jb + 1) * P],
                                    qpT[row0 : row0 + 64, oc, i0 + nb * 512 : i0 + (nb + 1) * 512],
                                    start=True,
                                    stop=True,
                                )
                            pT = probspool.tile([P, IG], F32R, tag="probsT")
                            nc.scalar.activation(pT[:], qk[:], EXP, scale=0.125)
                            for nb in range(IG // 512):
                                nc.tensor.matmul(
                                    av[:, nb * 512 : (nb + 1) * 512],
                                    vpo[h][:, jb, :],
                                    pT[:, nb * 512 : (nb + 1) * 512],
                                    start=(jb == 0),
                                    stop=(jb == NJB - 1),
                                )
                        # normalize: recip of denominator row, broadcast via K=1 matmul
                        bc = bcpool.tile([P, IG], F32R, tag="bc")
                        with nc.allow_low_precision(reason="f32r rounding"):
                            nc.vector.reciprocal(
                                bc[srow : srow + 1, :], av[srow : srow + 1, :]
                            )
                        bcp = psum.tile([P, IG], F32, tag="pA")
                        for nb in range(IG // 512):
                            nc.tensor.matmul(
                                bcp[:, nb * 512 : (nb + 1) * 512],
                                onesr[srow : srow + 1, :],
                                bc[srow : srow + 1, nb * 512 : (nb + 1) * 512],
                                start=True,
                                stop=True,
                            )
                        nc.any.tensor_copy(
                            bc[vrow0 : vrow0 + 64, :], bcp[vrow0 : vrow0 + 64, :]
                        )
                        nc.vector.tensor_tensor(
                            attT[vrow0 : vrow0 + 64, h // 2, i0 : i0 + IG],
                            av[vrow0 : vrow0 + 64, :],
                            bc[vrow0 : vrow0 + 64, :],
                            mybir.AluOpType.mult,
                        )

            # ---- output projection ----
            with nc.named_scope("wout"):
                for ib in range(N // P):
                    fin = psum.tile([P, D], F32, tag="pB")
                    for pc in range(2):
                        for nb in range(2):
                            nc.tensor.matmul(
                                fin[:, nb * 512 : (nb + 1) * 512],
                                attT[:, pc, ib * P : (ib + 1) * P],
                                woutr[:, pc, nb * 512 : (nb + 1) * 512],
                                start=(pc == 0),
                                stop=(pc == 1),
                            )
                    ot = opool.tile([P, D], F32, tag="ostage")
                    nc.any.tensor_copy(ot[:], fin[:])
                    nc.sync.dma_start(out[ib * P : (ib + 1) * P, :], ot[:])

    nc.compile()
    return nc


_NC = None


def _get_nc():
    global _NC
    if _NC is None:
        _NC = build()
    return _NC


def make_in_maps(q, kv, Wq, Wkv, Wout):
    q = np.ascontiguousarray(q, dtype=np.float32)
    kv = np.ascontiguousarray(kv, dtype=np.float32)
    Wq = np.ascontiguousarray(Wq, dtype=np.float32)
    Wkv = np.ascontiguousarray(Wkv, dtype=np.float32)
    Wout = np.ascontiguousarray(Wout, dtype=np.float32)
    in_maps = []
    for c in range(8):
        b, g = c // 4, c % 4
        sl = slice(g * O, (g + 1) * O)
        in_maps.append(
            {
                "xq": q[b],
                "xkv": kv[b],
                "wq": np.ascontiguousarray(Wq[:, sl]),
                "wk": np.ascontiguousarray(Wkv[:, sl]),
                "wv": np.ascontiguousarray(Wkv[:, D + g * O : D + (g + 1) * O]),
                "wout": np.ascontiguousarray(Wout[sl, :]),
            }
        )
    return in_maps


def gather(results):
    out = np.zeros((2, N, D), dtype=np.float32)
    for c in range(8):
        out[c // 4] += results[c]["out"]
    return out


def kernel(**inputs):
    nc = _get_nc()
    in_maps = make_in_maps(**inputs)
    res = run_bass_kernel_spmd(nc, in_maps, core_ids=list(range(8)))
    return gather(res.results)


if __name__ == "__main__":
    rng = np.random.default_rng(0)
    ins = {
        "q": rng.standard_normal((2, N, D), dtype=np.float32),
        "kv": rng.standard_normal((2, N, D), dtype=np.float32),
        "Wq": (rng.standard_normal((D, D), dtype=np.float32) / np.sqrt(D)).astype(np.float32),
        "Wkv": (rng.standard_normal((D, 2 * D), dtype=np.float32) / np.sqrt(D)).astype(np.float32),
        "Wout": (rng.standard_normal((D, D), dtype=np.float32) / np.sqrt(D)).astype(np.float32),
    }
    out = kernel(**ins)
    print("ok", out.shape, out.dtype)


# revision 20
# speedup vs baseline: 1.1992x; 1.1992x over previous
"""Trainium2 Bass kernel for CustomMHA (B=2, N=2048, D=1024, H=16, fp32).

Sharding: 8 cores = (batch b = core//4) x (head-group g = core%4, 4 heads each).
Each core computes, for its batch and its 4 heads:
    attn_out_heads @ Wout[rows of its heads]  ->  a partial [N, D] output.
Host sums the 4 partials per batch (Megatron-style row-parallel output).

Per-core pipeline (all matmuls in fp32r = TF32-like, fp32 accumulate):
  1. PE-transpose x_q, x_kv -> xT [D, N] slabs.
  2. Projections: qT/kT in [d_head, N] (transposed) layout, v in [N, d_head]
     natural layout with per-head ones/zeros-padded columns (vpo) so the
     AV matmul also produces the softmax denominator row.
  3. Attention per head, scoresT orientation [key-part, query-free]:
     QK matmul -> PSUM, exp on ACT (scale=1/8 folded) -> probsT (f32r),
     AV matmul accumulates over key blocks.  Denominator row broadcast via
     a K=1 matmul with a ones column; normalize with one DVE multiply.
  4. Row-sharded Wout matmul -> partial output.

PSUM layout (8 banks):
  psA  (2 slots x 2 banks): qk tiles (exp pipeline) + early-phase transposes
  psAV (1 slot  x 2 banks): AV accumulators
  psC  (1 slot  x 2 banks): background work -- late q slabs, projection
        accumulators, denominator broadcast, output projection
"""

import sys

sys.path.insert(0, "/opt/trn_rl_repo")

import numpy as np

import concourse.bass as bass
import concourse.mybir as mybir
import concourse.tile as tile
from concourse import bacc
from concourse.bass_utils import run_bass_kernel_spmd
from concourse.masks import make_identity

F32 = mybir.dt.float32
F32R = mybir.dt.float32r
EXP = mybir.ActivationFunctionType.Exp

N = 2048  # sequence length
D = 1024  # model dim
HL = 4    # heads per core
O = HL * 64  # per-core projection width (256)
P = 128
NSLAB = 512          # i/j rows per transpose+projection slab
NSLABS = N // NSLAB  # 4
IG = 1024            # attention query-column group
NJB = N // P         # 16 key blocks
DC = D // P          # 8 contraction chunks


def build():
    nc = bacc.Bacc("TRN2", debug=False, num_devices=8)
    xq = nc.dram_tensor("xq", [N, D], F32, kind="ExternalInput").ap()
    xkv = nc.dram_tensor("xkv", [N, D], F32, kind="ExternalInput").ap()
    wq = nc.dram_tensor("wq", [D, O], F32, kind="ExternalInput").ap()
    wk = nc.dram_tensor("wk", [D, O], F32, kind="ExternalInput").ap()
    wv = nc.dram_tensor("wv", [D, O], F32, kind="ExternalInput").ap()
    wout = nc.dram_tensor("wout", [O, D], F32, kind="ExternalInput").ap()
    out = nc.dram_tensor("out", [N, D], F32, kind="ExternalOutput").ap()

    with tile.TileContext(nc) as tc:
        with (
            tc.tile_pool(name="consts", bufs=1) as consts,
            tc.tile_pool(name="weights", bufs=1) as wpool,
            tc.tile_pool(name="wstage", bufs=1) as wstage,
            tc.tile_pool(name="xstage", bufs=3) as xstage,
            tc.tile_pool(name="xT", bufs=2) as xTpool,
            tc.tile_pool(name="proj", bufs=1) as projpool,
            tc.tile_pool(name="probs", bufs=6) as probspool,
            tc.tile_pool(name="bc", bufs=2) as bcpool,
            tc.tile_pool(name="ostage", bufs=2) as opool,
            tc.tile_pool(name="psA", bufs=2, space="PSUM") as psA,
            tc.tile_pool(name="psAV", bufs=1, space="PSUM") as psAV,
            tc.tile_pool(name="psC", bufs=1, space="PSUM") as psC,
        ):
            # ---- constants ----
            ident = consts.tile([P, P], F32)
            make_identity(nc, ident[:])
            onesf = consts.tile([P, P], F32)
            nc.vector.memset(onesf[:], 1.0)
            onesr = consts.tile([P, P], F32R)
            nc.vector.tensor_copy(onesr[:], onesf[:])
            # pat: [1, 0, 0, ...] column pattern for vpo padding halves
            pat = consts.tile([P, 64], F32)
            nc.vector.memset(pat[:, 0:1], 1.0)
            nc.vector.memset(pat[:, 1:64], 0.0)

            # ---- weights: DMA fp32, round to f32r ----
            with nc.named_scope("weights"):
                wr = {}
                for name, w in (("wq", wq), ("wk", wk), ("wv", wv)):
                    wt = wpool.tile([P, DC, O], F32R, tag=f"{name}r", name="wt")
                    wst = wstage.tile([P, DC, O], F32, tag="wstage", name="wst")
                    nc.sync.dma_start(wst[:], w.rearrange("(c p) o -> p c o", p=P))
                    nc.any.tensor_copy(wt[:], wst[:])
                    wr[name] = wt
                woutr = wpool.tile([P, 2, D], F32R, tag="woutr")
                wst = wstage.tile([P, 2, D], F32, tag="wstage", name="wst")
                nc.sync.dma_start(wst[:], wout.rearrange("(c p) o -> p c o", p=P))
                nc.any.tensor_copy(woutr[:], wst[:])

            # ---- persistent activations ----
            qpT = projpool.tile([P, 2, N], F32R, tag="qpT")
            kpT = projpool.tile([P, 2, N], F32R, tag="kpT")
            vpo = [
                projpool.tile([P, NJB, P], F32R, tag=f"vpo{h}", name=f"vpo{h}")
                for h in range(HL)
            ]
            attT = projpool.tile([P, 2, N], F32R, tag="attT")

            # vpo padding halves: ones column + zeros
            for h in range(HL):
                pad0 = 64 if h % 2 == 0 else 0
                nc.vector.tensor_copy(
                    vpo[h][:, :, pad0 : pad0 + 64],
                    pat[:, None, :].to_broadcast([P, NJB, 64]),
                )

            _proj_flip = [0]

            def proj_psum(width):
                """Alternate projection accumulators between the psAV and psC
                single-slot pools to get a 2-deep accumulate/copy pipeline."""
                _proj_flip[0] ^= 1
                pool = psAV if _proj_flip[0] else psC
                tag = "av" if pool is psAV else "c"
                t = pool.tile([P, IG], F32, tag=tag, name="pp")
                return t[:, :width]

            def emit_slab(chain, s, background=False):
                """Transpose one 512-row slab of xq/xkv and project it.

                background=True (late q slabs): run everything through the
                psC slot so the attention pipeline's psA/psAV rotations are
                never stalled; this work crawls along during attention.
                """
                xin = xkv if chain == "kv" else xq
                cp = nc.any.tensor_copy
                slab = xTpool.tile([P, DC, NSLAB], F32R, tag="xT", name="slab")
                for half in range(NSLAB // P):
                    ib = s * (NSLAB // P) + half
                    xt = xstage.tile([P, D], F32, tag="xstage", name="xt")
                    nc.sync.dma_start(xt[:], xin[ib * P : (ib + 1) * P, :])
                    if background:
                        quads = [psC.tile([P, IG], F32, tag="c", name="tp")[:, : DC * P]]
                    else:
                        quads = [psA.tile([P, DC * P], F32, tag="qk", name="tp")]
                    qw = DC * P // len(quads)
                    for qi, tp in enumerate(quads):
                        for dq in range(qw // P):
                            nc.tensor.transpose(
                                tp[:, dq * P : (dq + 1) * P],
                                xt[:, qi * qw + dq * P : qi * qw + (dq + 1) * P],
                                ident[:],
                            )
                        cp(
                            slab[:, (qi * qw) // P : (qi * qw + qw) // P, half * P : (half + 1) * P],
                            tp[:, :qw].rearrange("p (c i) -> p c i", c=qw // P),
                        )
                ssl = slice(s * NSLAB, (s + 1) * NSLAB)
                wname, dstT = ("wk", kpT) if chain == "kv" else ("wq", qpT)
                for oc in range(2):
                    if background:
                        ps = psC.tile([P, IG], F32, tag="c", name="ps")[:, :NSLAB]
                    else:
                        ps = proj_psum(NSLAB)
                    for dc in range(DC):
                        nc.tensor.matmul(
                            ps[:],
                            wr[wname][:, dc, oc * P : (oc + 1) * P],
                            slab[:, dc, :],
                            start=(dc == 0),
                            stop=(dc == DC - 1),
                        )
                    cp(dstT[:, oc, ssl], ps[:])
                if chain == "kv":
                    # v projection (natural layout) + scatter into vpo
                    for half in range(NSLAB // P):
                        jb = s * (NSLAB // P) + half
                        ps = proj_psum(O)
                        for dc in range(DC):
                            nc.tensor.matmul(
                                ps[:],
                                slab[:, dc, half * P : (half + 1) * P],
                                wr["wv"][:, dc, :],
                                start=(dc == 0),
                                stop=(dc == DC - 1),
                            )
                        for h in range(HL):
                            v0 = 0 if h % 2 == 0 else 64
                            cp(
                                vpo[h][:, jb, v0 : v0 + 64],
                                ps[:, h * 64 : (h + 1) * 64],
                            )

            def flush_av(carry):
                """Emit the deferred last AV pair of the previous group."""
                ph, pav, ppT = carry
                with tc.high_priority(offset=-30):
                    for nb in range(IG // 512):
                        nc.tensor.matmul(
                            pav[:, nb * 512 : (nb + 1) * 512],
                            vpo[ph][:, NJB - 1, :],
                            ppT[:, nb * 512 : (nb + 1) * 512],
                            start=False,
                            stop=True,
                        )

            def emit_attention(h, ig, at_jb0=None):
                oc, row0 = h // 2, (h % 2) * 64
                i0 = ig * IG
                av = psAV.tile([P, IG], F32, tag="av", name="av")
                # AV(jb) is emitted after QK(jb+1)/exp(jb+1) so the PE finishes
                # QK(jb+1) while exp(jb) runs; the final AV pair is carried into
                # the next group (flushed via at_jb0) so it never delays the
                # boundary exps.
                pend_pT = None
                for jb in range(NJB):
                    qk = psA.tile([P, IG], F32, tag="qk", name="qk")
                    for nb in range(IG // 512):
                        nc.tensor.matmul(
                            qk[:, nb * 512 : (nb + 1) * 512],
                            kpT[row0 : row0 + 64, oc, jb * P : (jb + 1) * P],
                            qpT[row0 : row0 + 64, oc, i0 + nb * 512 : i0 + (nb + 1) * 512],
                            start=True,
                            stop=True,
                        )
                    pT = probspool.tile([P, IG], F32R, tag="probsT", name="pT")
                    nc.scalar.activation(pT[:], qk[:], EXP, scale=0.125)
                    if jb == 0 and at_jb0 is not None:
                        at_jb0()
                    if pend_pT is not None:
                        pjb, ppT = pend_pT
                        with tc.high_priority(offset=-30):
                            for nb in range(IG // 512):
                                nc.tensor.matmul(
                                    av[:, nb * 512 : (nb + 1) * 512],
                                    vpo[h][:, pjb, :],
                                    ppT[:, nb * 512 : (nb + 1) * 512],
                                    start=(pjb == 0),
                                    stop=False,
                                )
                    pend_pT = (jb, pT)
                return av, (h, av, pend_pT[1])

            def emit_drain(h, ig, av):
                """Normalize group (h, ig); emitted one group late so the
                latency hides under the next group's j-loop.  Copy-first so
                the av PSUM slot is released after two DVE ops, then divide
                in place with the broadcast denominator still in PSUM."""
                vrow0 = (h % 2) * 64
                srow = 64 - vrow0
                i0 = ig * IG
                dst = attT[vrow0 : vrow0 + 64, h // 2, i0 : i0 + IG]
                nc.vector.tensor_copy(dst, av[vrow0 : vrow0 + 64, :])
                bc = bcpool.tile([P, IG], F32R, tag="bc", name="bc")
                with nc.allow_low_precision(reason="f32r rounding"):
                    nc.vector.reciprocal(bc[srow : srow + 1, :], av[srow : srow + 1, :])
                bcp = psC.tile([P, IG], F32, tag="c", name="bcp")
                for nb in range(IG // 512):
                    nc.tensor.matmul(
                        bcp[:, nb * 512 : (nb + 1) * 512],
                        onesr[srow : srow + 1, :],
                        bc[srow : srow + 1, nb * 512 : (nb + 1) * 512],
                        start=True,
                        stop=True,
                    )
                nc.vector.tensor_tensor(
                    dst, dst, bcp[vrow0 : vrow0 + 64, :], mybir.AluOpType.mult
                )

            def emit_wout(ib, pool, tag, early=False):
                fin = pool.tile([P, D], F32, tag=tag, name="fin")
                for pc in range(2):
                    for nb in range(2):
                        nc.tensor.matmul(
                            fin[:, nb * 512 : (nb + 1) * 512],
                            attT[:, pc, ib * P : (ib + 1) * P],
                            woutr[:, pc, nb * 512 : (nb + 1) * 512],
                            start=(pc == 0),
                            stop=(pc == 1),
                        )
                ot = opool.tile([P, D], F32, tag="ostage", name="ot")
                cpf = nc.vector.tensor_copy if early else (
                    nc.scalar.copy if ib % 2 == 0 else nc.vector.tensor_copy
                )
                cpf(ot[:], fin[:])
                nc.sync.dma_start(out[ib * P : (ib + 1) * P, :], ot[:])

            # ---- emission order ----
            with nc.named_scope("proj"):
                for s in range(NSLABS):
                    emit_slab("kv", s)
                emit_slab("q", 0)
                emit_slab("q", 1)

            with nc.named_scope("attention"):
                groups = [(h, 0) for h in range(HL)] + [(h, 1) for h in range(HL)]
                carry = None
                pend_drain = None
                for gi, (h, ig) in enumerate(groups):
                    pc, pd = carry, pend_drain

                    def at_jb0(pc=pc, pd=pd):
                        if pc is not None:
                            flush_av(pc)
                        if pd is not None:
                            emit_drain(*pd)

                    av, carry = emit_attention(h, ig, at_jb0)
                    pend_drain = (h, ig, av)
                    if gi == 0:
                        with nc.named_scope("proj2"):
                            emit_slab("q", 2, background=True)
                    elif gi == 1:
                        with nc.named_scope("proj3"):
                            emit_slab("q", 3, background=True)
                    elif gi == 5:
                        # ig=0 halves of attT are final: first 8 output blocks
                        # crawl through the psC slot during ig=1 attention
                        with nc.named_scope("wout_early"), tc.high_priority(offset=-(10**6)):
                            for ib in range(N // P // 2):
                                emit_wout(ib, psC, "c", early=True)
                flush_av(carry)
                emit_drain(*pend_drain)

            # ---- output projection (second half) ----
            with nc.named_scope("wout"):
                rot = [(psA, "qk"), (psAV, "av"), (psA, "qk"), (psC, "c")]
                for ib in range(N // P // 2, N // P):
                    pool, tag = rot[ib % 4]
                    emit_wout(ib, pool, tag)

    nc.compile()
    return nc


_NC = None


def _get_nc():
    global _NC
    if _NC is None:
        _NC = build()
    return _NC


def make_in_maps(q, kv, Wq, Wkv, Wout):
    q = np.ascontiguousarray(q, dtype=np.float32)
    kv = np.ascontiguousarray(kv, dtype=np.float32)
    Wq = np.ascontiguousarray(Wq, dtype=np.float32)
    Wkv = np.ascontiguousarray(Wkv, dtype=np.float32)
    Wout = np.ascontiguousarray(Wout, dtype=np.float32)
    in_maps = []
    for c in range(8):
        b, g = c // 4, c % 4
        sl = slice(g * O, (g + 1) * O)
        in_maps.append(
            {
                "xq": q[b],
                "xkv": kv[b],
                "wq": np.ascontiguousarray(Wq[:, sl]),
                "wk": np.ascontiguousarray(Wkv[:, sl]),
                "wv": np.ascontiguousarray(Wkv[:, D + g * O : D + (g + 1) * O]),
                "wout": np.ascontiguousarray(Wout[sl, :]),
            }
        )
    return in_maps


def gather(results):
    out = np.zeros((2, N, D), dtype=np.float32)
    for c in range(8):
        out[c // 4] += results[c]["out"]
    return out


def kernel(**inputs):
    nc = _get_nc()
    in_maps = make_in_maps(**inputs)
    res = run_bass_kernel_spmd(nc, in_maps, core_ids=list(range(8)))
    return gather(res.results)


if __name__ == "__main__":
    rng = np.random.default_rng(0)
    ins = {
        "q": rng.standard_normal((2, N, D), dtype=np.float32),
        "kv": rng.standard_normal((2, N, D), dtype=np.float32),
        "Wq": (rng.standard_normal((D, D), dtype=np.float32) / np.sqrt(D)).astype(np.float32),
        "Wkv": (rng.standard_normal((D, 2 * D), dtype=np.float32) / np.sqrt(D)).astype(np.float32),
        "Wout": (rng.standard_normal((D, D), dtype=np.float32) / np.sqrt(D)).astype(np.float32),
    }
    out = kernel(**ins)
    print("ok", out.shape, out.dtype)


# revision 24
# speedup vs baseline: 16696.6251x; 13922.6308x over previous
"""Trainium2 Bass kernel for CustomMHA (B=2, N=2048, D=1024, H=16, fp32).

Sharding: 8 cores = (batch b = core//4) x (head-group g = core%4, 4 heads each).
Each core computes, for its batch and its 4 heads:
    attn_out_heads @ Wout[rows of its heads]  ->  a partial [N, D] output.
Host sums the 4 partials per batch (Megatron-style row-parallel output).

Per-core pipeline (all matmuls in fp32r = TF32-like, fp32 accumulate):
  1. PE-transpose x_q, x_kv -> xT [D, N] slabs.
  2. Projections: qT/kT in [d_head, N] (transposed) layout, v in [N, d_head]
     natural layout with per-head ones/zeros-padded columns (vpo) so the
     AV matmul also produces the softmax denominator row.
  3. Attention per head, scoresT orientation [key-part, query-free]:
     QK matmul -> PSUM, exp on ACT (scale=1/8 folded) -> probsT (f32r),
     AV matmul accumulates over key blocks.  Denominator row broadcast via
     a K=1 matmul with a ones column; normalize with one DVE multiply.
  4. Row-sharded Wout matmul -> partial output.

PSUM layout (8 banks):
  psA  (2 slots x 2 banks): qk tiles (exp pipeline) + early-phase transposes
  psAV (1 slot  x 2 banks): AV accumulators
  psC  (1 slot  x 2 banks): background work -- late q slabs, projection
        accumulators, denominator broadcast, output projection
"""

import sys

sys.path.insert(0, "/opt/trn_rl_repo")

import numpy as np

import concourse.bass as bass
import concourse.mybir as mybir
import concourse.tile as tile
from concourse import bacc
from concourse.bass_utils import run_bass_kernel_spmd
from concourse.masks import make_identity

F32 = mybir.dt.float32
F32R = mybir.dt.float32r
EXP = mybir.ActivationFunctionType.Exp

N = 2048  # sequence length
D = 1024  # model dim
HL = 4    # heads per core
O = HL * 64  # per-core projection width (256)
P = 128
NSLAB = 512          # i/j rows per transpose+projection slab
NSLABS = N // NSLAB  # 4
IG = 1024            # attention query-column group
NJB = N // P         # 16 key blocks
DC = D // P          # 8 contraction chunks


def build():
    nc = bacc.Bacc("TRN2", debug=False, num_devices=8)
    xq = nc.dram_tensor("xq", [N, D], F32, kind="ExternalInput").ap()
    xkv = nc.dram_tensor("xkv", [N, D], F32, kind="ExternalInput").ap()
    wq = nc.dram_tensor("wq", [D, O], F32, kind="ExternalInput").ap()
    wk = nc.dram_tensor("wk", [D, O], F32, kind="ExternalInput").ap()
    wv = nc.dram_tensor("wv", [D, O], F32, kind="ExternalInput").ap()
    wout = nc.dram_tensor("wout", [O, D], F32, kind="ExternalInput").ap()
    out = nc.dram_tensor("out", [N, D], F32, kind="ExternalOutput").ap()

    with tile.TileContext(nc) as tc:
        with (
            tc.tile_pool(name="consts", bufs=1) as consts,
            tc.tile_pool(name="weights", bufs=1) as wpool,
            tc.tile_pool(name="wstage", bufs=1) as wstage,
            tc.tile_pool(name="xstage", bufs=3) as xstage,
            tc.tile_pool(name="xT", bufs=2) as xTpool,
            tc.tile_pool(name="proj", bufs=1) as projpool,
            tc.tile_pool(name="probs", bufs=6) as probspool,
            tc.tile_pool(name="bc", bufs=2) as bcpool,
            tc.tile_pool(name="ostage", bufs=2) as opool,
            tc.tile_pool(name="psA", bufs=2, space="PSUM") as psA,
            tc.tile_pool(name="psAV", bufs=1, space="PSUM") as psAV,
            tc.tile_pool(name="psC", bufs=1, space="PSUM") as psC,
        ):
            # ---- constants ----
            ident = consts.tile([P, P], F32)
            make_identity(nc, ident[:])
            onesf = consts.tile([P, P], F32)
            nc.vector.memset(onesf[:], 1.0)
            onesr = consts.tile([P, P], F32R)
            nc.vector.tensor_copy(onesr[:], onesf[:])
            # pat: [1, 0, 0, ...] column pattern for vpo padding halves
            pat = consts.tile([P, 64], F32)
            nc.vector.memset(pat[:, 0:1], 1.0)
            nc.vector.memset(pat[:, 1:64], 0.0)

            # ---- weights: DMA fp32, round to f32r ----
            with nc.named_scope("weights"):
                wr = {}
                for name, w in (("wq", wq), ("wk", wk), ("wv", wv)):
                    wt = wpool.tile([P, DC, O], F32R, tag=f"{name}r", name="wt")
                    wst = wstage.tile([P, DC, O], F32, tag="wstage", name="wst")
                    nc.sync.dma_start(wst[:], w.rearrange("(c p) o -> p c o", p=P))
                    nc.any.tensor_copy(wt[:], wst[:])
                    wr[name] = wt
                woutr = wpool.tile([P, 2, D], F32R, tag="woutr")
                wst = wstage.tile([P, 2, D], F32, tag="wstage", name="wst")
                nc.sync.dma_start(wst[:], wout.rearrange("(c p) o -> p c o", p=P))
                nc.any.tensor_copy(woutr[:], wst[:])

            # ---- persistent activations ----
            qpT = projpool.tile([P, 2, N], F32R, tag="qpT")
            kpT = projpool.tile([P, 2, N], F32R, tag="kpT")
            vpo = [
                projpool.tile([P, NJB, P], F32R, tag=f"vpo{h}", name=f"vpo{h}")
                for h in range(HL)
            ]
            attT = projpool.tile([P, 2, N], F32R, tag="attT")

            # vpo padding halves: ones column + zeros
            for h in range(HL):
                pad0 = 64 if h % 2 == 0 else 0
                nc.vector.tensor_copy(
                    vpo[h][:, :, pad0 : pad0 + 64],
                    pat[:, None, :].to_broadcast([P, NJB, 64]),
                )

            _proj_flip = [0]

            def proj_psum(width):
                """Alternate projection accumulators between the psAV and psC
                single-slot pools to get a 2-deep accumulate/copy pipeline."""
                _proj_flip[0] ^= 1
                pool = psAV if _proj_flip[0] else psC
                tag = "av" if pool is psAV else "c"
                t = pool.tile([P, IG], F32, tag=tag, name="pp")
                return t[:, :width]

            def emit_slab(chain, s, background=False):
                """Transpose one 512-row slab of xq/xkv and project it.

                background=True (late q slabs): run everything through the
                psC slot so the attention pipeline's psA/psAV rotations are
                never stalled; this work crawls along during attention.
                """
                xin = xkv if chain == "kv" else xq
                cp = nc.any.tensor_copy
                slab = xTpool.tile([P, DC, NSLAB], F32R, tag="xT", name="slab")
                for half in range(NSLAB // P):
                    ib = s * (NSLAB // P) + half
                    xt = xstage.tile([P, D], F32, tag="xstage", name="xt")
                    nc.sync.dma_start(xt[:], xin[ib * P : (ib + 1) * P, :])
                    if background:
                        quads = [psC.tile([P, IG], F32, tag="c", name="tp")[:, : DC * P]]
                    else:
                        quads = [psA.tile([P, DC * P], F32, tag="qk", name="tp")]
                    qw = DC * P // len(quads)
                    for qi, tp in enumerate(quads):
                        for dq in range(qw // P):
                            nc.tensor.transpose(
                                tp[:, dq * P : (dq + 1) * P],
                                xt[:, qi * qw + dq * P : qi * qw + (dq + 1) * P],
                                ident[:],
                            )
                        cp(
                            slab[:, (qi * qw) // P : (qi * qw + qw) // P, half * P : (half + 1) * P],
                            tp[:, :qw].rearrange("p (c i) -> p c i", c=qw // P),
                        )
                ssl = slice(s * NSLAB, (s + 1) * NSLAB)
                wname, dstT = ("wk", kpT) if chain == "kv" else ("wq", qpT)
                for oc in range(2):
                    if background:
                        ps = psC.tile([P, IG], F32, tag="c", name="ps")[:, :NSLAB]
                    else:
                        ps = proj_psum(NSLAB)
                    for dc in range(DC):
                        nc.tensor.matmul(
                            ps[:],
                            wr[wname][:, dc, oc * P : (oc + 1) * P],
                            slab[:, dc, :],
                            start=(dc == 0),
                            stop=(dc == DC - 1),
                        )
                    cp(dstT[:, oc, ssl], ps[:])
                if chain == "kv":
                    # v projection (natural layout) + scatter into vpo
                    for half in range(NSLAB // P):
                        jb = s * (NSLAB // P) + half
                        ps = proj_psum(O)
                        for dc in range(DC):
                            nc.tensor.matmul(
                                ps[:],
                                slab[:, dc, half * P : (half + 1) * P],
                                wr["wv"][:, dc, :],
                                start=(dc == 0),
                                stop=(dc == DC - 1),
                            )
                        for h in range(HL):
                            v0 = 0 if h % 2 == 0 else 64
                            cp(
                                vpo[h][:, jb, v0 : v0 + 64],
                                ps[:, h * 64 : (h + 1) * 64],
                            )

            def flush_av(carry):
                """Emit the deferred last AV pair of the previous group."""
                ph, pav, ppT = carry
                with tc.high_priority(offset=-30):
                    for nb in range(IG // 512):
                        nc.tensor.matmul(
                            pav[:, nb * 512 : (nb + 1) * 512],
                            vpo[ph][:, NJB - 1, :],
                            ppT[:, nb * 512 : (nb + 1) * 512],
                            start=False,
                            stop=True,
                        )

            def emit_attention(h, ig, at_jb0=None):
                oc, row0 = h // 2, (h % 2) * 64
                i0 = ig * IG
                av = psAV.tile([P, IG], F32, tag="av", name="av")
                # AV(jb) is emitted after QK(jb+1)/exp(jb+1) so the PE finishes
                # QK(jb+1) while exp(jb) runs; the final AV pair is carried into
                # the next group (flushed via at_jb0) so it never delays the
                # boundary exps.
                pend_pT = None
                for jb in range(NJB):
                    qk = psA.tile([P, IG], F32, tag="qk", name="qk")
                    for nb in range(IG // 512):
                        nc.tensor.matmul(
                            qk[:, nb * 512 : (nb + 1) * 512],
                            kpT[row0 : row0 + 64, oc, jb * P : (jb + 1) * P],
                            qpT[row0 : row0 + 64, oc, i0 + nb * 512 : i0 + (nb + 1) * 512],
                            start=True,
                            stop=True,
                        )
                    pT = probspool.tile([P, IG], F32R, tag="probsT", name="pT")
                    nc.scalar.activation(pT[:], qk[:], EXP, scale=0.125)
                    if jb == 0 and at_jb0 is not None:
                        at_jb0()
                    if pend_pT is not None:
                        pjb, ppT = pend_pT
                        with tc.high_priority(offset=-30):
                            for nb in range(IG // 512):
                                nc.tensor.matmul(
                                    av[:, nb * 512 : (nb + 1) * 512],
                                    vpo[h][:, pjb, :],
                                    ppT[:, nb * 512 : (nb + 1) * 512],
                                    start=(pjb == 0),
                                    stop=False,
                                )
                    pend_pT = (jb, pT)
                return av, (h, av, pend_pT[1])

            def emit_drain(h, ig, av):
                """Normalize group (h, ig); emitted one group late so the
                latency hides under the next group's j-loop.  Copy-first so
                the av PSUM slot is released after two DVE ops, then divide
                in place with the broadcast denominator still in PSUM."""
                vrow0 = (h % 2) * 64
                srow = 64 - vrow0
                i0 = ig * IG
                dst = attT[vrow0 : vrow0 + 64, h // 2, i0 : i0 + IG]
                nc.vector.tensor_copy(dst, av[vrow0 : vrow0 + 64, :])
                bc = bcpool.tile([P, IG], F32R, tag="bc", name="bc")
                with nc.allow_low_precision(reason="f32r rounding"):
                    nc.vector.reciprocal(bc[srow : srow + 1, :], av[srow : srow + 1, :])
                bcp = psC.tile([P, IG], F32, tag="c", name="bcp")
                for nb in range(IG // 512):
                    nc.tensor.matmul(
                        bcp[:, nb * 512 : (nb + 1) * 512],
                        onesr[srow : srow + 1, :],
                        bc[srow : srow + 1, nb * 512 : (nb + 1) * 512],
                        start=True,
                        stop=True,
                    )
                nc.vector.tensor_tensor(
                    dst, dst, bcp[vrow0 : vrow0 + 64, :], mybir.AluOpType.mult
                )

            def emit_wout(ib, pool, tag, early=False):
                fin = pool.tile([P, D], F32, tag=tag, name="fin")
                for pc in range(2):
                    for nb in range(2):
                        nc.tensor.matmul(
                            fin[:, nb * 512 : (nb + 1) * 512],
                            attT[:, pc, ib * P : (ib + 1) * P],
                            woutr[:, pc, nb * 512 : (nb + 1) * 512],
                            start=(pc == 0),
                            stop=(pc == 1),
                        )
                ot = opool.tile([P, D], F32, tag="ostage", name="ot")
                cpf = nc.vector.tensor_copy if early else (
                    nc.scalar.copy if ib % 2 == 0 else nc.vector.tensor_copy
                )
                cpf(ot[:], fin[:])
                nc.sync.dma_start(out[ib * P : (ib + 1) * P, :], ot[:])

            # ---- emission order ----
            with nc.named_scope("proj"):
                for s in range(NSLABS):
                    emit_slab("kv", s)
                emit_slab("q", 0)
                emit_slab("q", 1)

            with nc.named_scope("attention"):
                groups = [(h, 0) for h in range(HL)] + [(h, 1) for h in range(HL)]
                carry = None
                pend_drain = None
                for gi, (h, ig) in enumerate(groups):
                    pc, pd = carry, pend_drain

                    def at_jb0(pc=pc, pd=pd):
                        if pc is not None:
                            flush_av(pc)
                        if pd is not None:
                            emit_drain(*pd)

                    av, carry = emit_attention(h, ig, at_jb0)
                    pend_drain = (h, ig, av)
                    if gi == 0:
                        with nc.named_scope("proj2"):
                            emit_slab("q", 2, background=True)
                    elif gi == 1:
                        with nc.named_scope("proj3"):
                            emit_slab("q", 3, background=True)
                    elif gi == 5:
                        # ig=0 halves of attT are final: first 8 output blocks
                        # crawl through the psC slot during ig=1 attention
                        with nc.named_scope("wout_early"), tc.high_priority(offset=-(10**6)):
                            for ib in range(N // P // 2):
                                emit_wout(ib, psC, "c", early=True)
                flush_av(carry)
                emit_drain(*pend_drain)

            # ---- output projection (second half) ----
            with nc.named_scope("wout"):
                rot = [(psA, "qk"), (psAV, "av"), (psA, "qk"), (psC, "c")]
                for ib in range(N // P // 2, N // P):
                    pool, tag = rot[ib % 4]
                    emit_wout(ib, pool, tag)

    nc.compile()
    return nc


_NC = None


def _get_nc():
    global _NC
    if _NC is None:
        _NC = build()
    return _NC


def make_in_maps(q, kv, Wq, Wkv, Wout):
    q = np.ascontiguousarray(q, dtype=np.float32)
    kv = np.ascontiguousarray(kv, dtype=np.float32)
    Wq = np.ascontiguousarray(Wq, dtype=np.float32)
    Wkv = np.ascontiguousarray(Wkv, dtype=np.float32)
    Wout = np.ascontiguousarray(Wout, dtype=np.float32)
    in_maps = []
    for c in range(8):
        b, g = c // 4, c % 4
        sl = slice(g * O, (g + 1) * O)
        in_maps.append(
            {
                "xq": q[b],
                "xkv": kv[b],
                "wq": np.ascontiguousarray(Wq[:, sl]),
                "wk": np.ascontiguousarray(Wkv[:, sl]),
                "wv": np.ascontiguousarray(Wkv[:, D + g * O : D + (g + 1) * O]),
                "wout": np.ascontiguousarray(Wout[sl, :]),
            }
        )
    return in_maps


def gather(results):
    out = np.zeros((2, N, D), dtype=np.float32)
    for c in range(8):
        out[c // 4] += results[c]["out"]
    return out


def kernel(**inputs):
    nc = _get_nc()
    in_maps = make_in_maps(**inputs)
    res = run_bass_kernel_spmd(nc, in_maps, core_ids=list(range(8)))
    return gather(res.results)


if __name__ == "__main__":
    rng = np.random.default_rng(0)
    ins = {
        "q": rng.standard_normal((2, N, D), dtype=np.float32),
        "kv": rng.standard_normal((2, N, D), dtype=np.float32),
        "Wq": (rng.standard_normal((D, D), dtype=np.float32) / np.sqrt(D)).astype(np.float32),
        "Wkv": (rng.standard_normal((D, 2 * D), dtype=np.float32) / np.sqrt(D)).astype(np.float32),
        "Wout": (rng.standard_normal((D, D), dtype=np.float32) / np.sqrt(D)).astype(np.float32),
    }
    out = kernel(**ins)
    print("ok", out.shape, out.dtype)
